# revision 24
# baseline (speedup 1.0000x reference)
"""Trainium2 Bass kernel for cross-modal channel-attention fusion (CCDPA).

Math (per batch b):
  pooled[c,m,d] = mean_{w,h} x_m[b,c,d,w,h]
  q = Wq @ pooled[:,0,:] + bq ; k_m = Wk @ pooled[:,m,:] + bk
  a[c,m] = softmax_m(q[c]·k_m[c] / sqrt(D))
  out[b,o,s] = sum_m a[o,m] * (Wc[m] @ x_m[b,:,s] + bc[m,o])
             = sum_m (a[o,m]*Wc[m,o,:]) @ x_m[b,:,s]  + sum_m a[o,m]*bc[m,o]

Sharding: 8 cores = (batch b = p//2) x (d-half = p%2).

Mixed precision: the pooling pass reads a separate fp8(e4m3) copy of the
shard (the attention logits are linear in pooled, so fp8 quantization of a
4M-element mean is far inside the softmax's noise floor), while the GEMM
pass reads a bf16 copy and accumulates in f32 PSUM; the output is stored
bf16 and widened on host.  This cuts HBM traffic from 144 MiB/core (f32
twice + f32 out) to 56 MiB/core (16 fp8 + 32 bf16 + 8 out).

Pooling sums are computed with DVE tensor_tensor_reduce (the fused
(a+b)->reduce form consumes two elements per ALU cycle, 2x the rate of a
plain reduce) plus a Scalar-engine share via activation(accum_out), so the
pooling pass stays close to DMA-bound.  The pooled-sum AllGather with the
partner core is split into two halves so the first one's ~20us latency
hides under the second half of the pooling pass.

The 1/(W*H) pooling mean and the 1/sqrt(D) logit scale are folded into the
Wq/Wk weights host-side, and bq/bk ride along as an extra contraction row
(augmented [D+1, D] weight matrices against pooled vectors with an appended
ones-row), so the device math needs no extra scaling ops.
"""

from contextlib import ExitStack

import numpy as np
import ml_dtypes

import concourse.bacc as bacc
import concourse.bass as bass
import concourse.mybir as mybir
import concourse.tile as tile
from concourse.bass_utils import run_bass_kernel_spmd

F32 = mybir.dt.float32
BF16 = mybir.dt.bfloat16
FP8 = mybir.dt.float8e4

NP_BF16 = ml_dtypes.bfloat16
NP_FP8 = ml_dtypes.float8_e4m3

B, C, D, W, H = 4, 256, 32, 32, 32
NCORES = 8
DHALF = D // 2  # d-slices per core
WH = W * H  # spatial elements per d-slice
S = DHALF * WH  # free elements per core shard


def _emit_program(nc, wh=WH, dhalf=DHALF):
    """Emit the SPMD per-core program. Identical on all 8 cores; per-core
    behavior comes only from per-core input data."""
    f32 = F32
    s = dhalf * wh
    dd = 2 * dhalf  # full D for this (possibly scaled-down) config
    nw = min(512, wh)  # matmul moving-dim chunk
    n_nh = wh // nw
    AX = mybir.AxisListType.X
    AF = mybir.ActivationFunctionType
    ALU = mybir.AluOpType

    # DVE (reduce_sum, ~1.06us/slice) vs ACT (activation accum,
    # ~1.23us/slice) share of the dhalf pooling slices per (m, ci) tile.
    dsplit = max(1, min(dhalf, int(round(dhalf * 9 / 16))))
    # pass-2 d-slices per block (2*DG PSUM banks live per oi pass)
    DG = 3

    x8s = [nc.dram_tensor(f"x8_{m}", [C, s], FP8, kind="ExternalInput") for m in range(4)]
    xbs = [nc.dram_tensor(f"xb_{m}", [C, s], BF16, kind="ExternalInput") for m in range(4)]
    wqT_d = nc.dram_tensor("wqTaug", [dd + 1, dd], f32, kind="ExternalInput")
    wkT_d = nc.dram_tensor("wkTaug", [dd + 1, dd], f32, kind="ExternalInput")
    wc_d = nc.dram_tensor("wc", [4, C, C], f32, kind="ExternalInput")
    bcT_d = nc.dram_tensor("bcT", [C, 4], f32, kind="ExternalInput")
    id_d = nc.dram_tensor("ident", [128, 128], f32, kind="ExternalInput")
    out_d = nc.dram_tensor("out", [C, s], BF16, kind="ExternalOutput")

    with tile.TileContext(nc) as tc, ExitStack() as ctx:
        const = ctx.enter_context(tc.tile_pool(name="const", bufs=1))
        pool1 = ctx.enter_context(tc.tile_pool(name="pass1", bufs=4))
        ascr = ctx.enter_context(tc.tile_pool(name="ascr", bufs=4))
        vscr = ctx.enter_context(tc.tile_pool(name="vscr", bufs=4))
        pool2 = ctx.enter_context(tc.tile_pool(name="pass2", bufs=48))
        outp = ctx.enter_context(tc.tile_pool(name="outp", bufs=6))
        attn = ctx.enter_context(tc.tile_pool(name="attn", bufs=1))
        scr = ctx.enter_context(tc.tile_pool(name="scr", bufs=2))
        psA = ctx.enter_context(tc.tile_pool(name="psA", bufs=2, space="PSUM"))
        psM = ctx.enter_context(tc.tile_pool(name="psM", bufs=6, space="PSUM"))
        dramp = ctx.enter_context(tc.tile_pool(name="dramp", bufs=1, space="DRAM"))

        # ---- constant loads (off critical path) ----
        ident = const.tile([128, 128], f32, tag="ident", name="ident")
        nc.sync.dma_start(out=ident[:], in_=id_d[:])
        wqT = const.tile([dd + 1, dd], f32, tag="wqT", name="wqT")
        nc.sync.dma_start(out=wqT[:], in_=wqT_d[:])
        wkT = const.tile([dd + 1, dd], f32, tag="wkT", name="wkT")
        nc.sync.dma_start(out=wkT[:], in_=wkT_d[:])
        wc_sb = []
        for oi in range(2):
            t = const.tile([128, 4 * C], f32, tag=f"wc{oi}", name=f"wc{oi}")
            for m in range(4):
                nc.sync.dma_start(
                    out=t[:, m * C : (m + 1) * C],
                    in_=wc_d[m, oi * 128 : (oi + 1) * 128, :],
                )
            wc_sb.append(t)
        bc_sb = []
        for oi in range(2):
            t = const.tile([128, 4], f32, tag=f"bc{oi}", name=f"bc{oi}")
            nc.sync.dma_start(out=t[:], in_=bcT_d[oi * 128 : (oi + 1) * 128, :])
            bc_sb.append(t)

        # ---- pass 1: pooling sums over (w,h) for each (c, m, d), fp8 copy ----
        # Half-shard tiles; DVE (3D reduce) and ACT (activation accum_out)
        # alternate 4/5 vs 4/3 d-slices per half so both engines stay
        # balanced (~1.06us vs ~1.23us per slice).
        nsl = max(1, dhalf // 2)  # d-slices per half-tile
        praw = [attn.tile([128, 4 * dhalf], f32, tag=f"praw{k}", name=f"praw{k}") for k in range(2)]
        for m in range(4):
            for ci in range(2):
                for hh in range(2):
                    t = pool1.tile([128, nsl * wh], FP8, tag="x1", name="x1")
                    nc.sync.dma_start(
                        out=t[:],
                        in_=x8s[m][
                            ci * 128 : (ci + 1) * 128,
                            hh * nsl * wh : (hh + 1) * nsl * wh,
                        ],
                    )
                    base = m * dhalf + hh * nsl
                    # DVE (fused (lo+hi)+reduce STT, ~0.6us/slice) vs ACT
                    # (activation accum, ~1.23us/slice) split, alternating
                    # so both engines stay balanced.
                    vh = max(1, (nsl * 11 + ((ci + hh) % 2) * 8) // 16)
                    vh = min(vh, nsl)
                    ah = nsl - vh
                    for d in range(vh):
                        vk = vscr.tile([128, wh // 2], BF16, tag="vscr", name="vscr")
                        nc.vector.scalar_tensor_tensor(
                            out=vk[:],
                            in0=t[:, d * wh : d * wh + wh // 2],
                            scalar=0.0,
                            in1=t[:, d * wh + wh // 2 : (d + 1) * wh],
                            op0=ALU.add,
                            op1=ALU.add,
                            accum_out=praw[ci][:, base + d : base + d + 1],
                        )
                    for d in range(vh, vh + ah):
                        sk = ascr.tile([128, wh], BF16, tag="ascr", name="ascr")
                        nc.scalar.activation(
                            sk[:],
                            t[:, d * wh : (d + 1) * wh],
                            AF.Copy,
                            accum_out=praw[ci][:, base + d : base + d + 1],
                        )

        # ---- exchange pooled halves with the partner core (two pipelined
        # AllGathers: m={0,1} issued while m={2,3} pooling still runs) ----
        hw = 2 * dhalf  # praw columns per collective half
        cc_in = [
            dramp.tile([C, hw], f32, tag=f"cc_in{g}", name=f"cc_in{g}")
            for g in range(2)
        ]
        cc_out = [
            dramp.tile([2 * C, hw], f32, tag=f"cc_out{g}", name=f"cc_out{g}")
            for g in range(2)
        ]
        for g in range(2):
            for ci in range(2):
                nc.sync.dma_start(
                    out=cc_in[g][ci * 128 : (ci + 1) * 128, :],
                    in_=praw[ci][:, g * hw : (g + 1) * hw],
                )
            nc.gpsimd.collective_compute(
                "AllGather",
                mybir.AluOpType.bypass,
                replica_groups=[[0, 1], [2, 3], [4, 5], [6, 7]],
                ins=[cc_in[g].opt()],
                outs=[cc_out[g].opt()],
            )
        # pooled_sb[k][c_local, m*D + d_global]
        pooled = [attn.tile([128, 4 * 2 * dhalf], f32, tag=f"pool{k}", name=f"pool{k}") for k in range(2)]
        for k in range(2):
            for h in range(2):
                for m in range(4):
                    g, mg = divmod(m, 2)
                    nc.sync.dma_start(
                        out=pooled[k][
                            :, m * 2 * dhalf + h * dhalf : m * 2 * dhalf + (h + 1) * dhalf
                        ],
                        in_=cc_out[g][
                            h * C + k * 128 : h * C + (k + 1) * 128,
                            mg * dhalf : (mg + 1) * dhalf,
                        ],
                    )

        # ---- attention weights ----
        # PTaug[m]: [D+1, 256] = pooled sums transposed, plus a ones-row
        ptaug = [attn.tile([dd + 1, C], f32, tag=f"pt{m}", name=f"pt{m}") for m in range(4)]
        for m in range(4):
            nc.vector.memset(ptaug[m][:], 1.0)
            for k in range(2):
                pst = psA.tile([dd, 128], f32, tag="att", name="att")
                nc.tensor.transpose(
                    pst[:], pooled[k][:, m * dd : (m + 1) * dd], ident[:]
                )
                nc.vector.tensor_copy(ptaug[m][0:dd, k * 128 : (k + 1) * 128], pst[:])
        qc = []
        kcs = [[None] * 2 for _ in range(4)]
        for k in range(2):
            psq = psA.tile([128, dd], f32, tag="att", name="att")
            nc.tensor.matmul(
                psq[:], lhsT=ptaug[0][:, k * 128 : (k + 1) * 128], rhs=wqT[:],
                start=True, stop=True,
            )
            t = attn.tile([128, dd], f32, tag=f"qc{k}", name=f"qc{k}")
            nc.vector.tensor_copy(t[:], psq[:])
            qc.append(t)
            for m in range(4):
                psk = psA.tile([128, dd], f32, tag="att", name="att")
                nc.tensor.matmul(
                    psk[:], lhsT=ptaug[m][:, k * 128 : (k + 1) * 128], rhs=wkT[:],
                    start=True, stop=True,
                )
                tk = attn.tile([128, dd], f32, tag=f"kc{m}_{k}", name=f"kc{m}_{k}")
                nc.vector.tensor_copy(tk[:], psk[:])
                kcs[m][k] = tk
        # logits (fused q*k -> sum) + softmax over m (free dim, 4 wide)
        a_sb = []
        for k in range(2):
            lg = attn.tile([128, 4], f32, tag=f"lg{k}", name=f"lg{k}")
            for m in range(4):
                sc = scr.tile([128, dd], f32, tag="ttr", name="ttr")
                nc.vector.tensor_mul(sc[:], qc[k][:], kcs[m][k][:])
                nc.vector.reduce_sum(out=lg[:, m : m + 1], in_=sc[:], axis=AX)
            mx = attn.tile([128, 1], f32, tag=f"mx{k}", name=f"mx{k}")
            nc.vector.reduce_max(out=mx[:], in_=lg[:], axis=AX)
            nc.vector.tensor_scalar_sub(out=lg[:], in0=lg[:], scalar1=mx[:])
            ex = attn.tile([128, 4], f32, tag=f"ex{k}", name=f"ex{k}")
            nc.scalar.activation(ex[:], lg[:], AF.Exp)
            sm = attn.tile([128, 1], f32, tag=f"sm{k}", name=f"sm{k}")
            nc.vector.reduce_sum(out=sm[:], in_=ex[:], axis=AX)
            rc = attn.tile([128, 1], f32, tag=f"rc{k}", name=f"rc{k}")
            nc.vector.reciprocal(out=rc[:], in_=sm[:])
            at = attn.tile([128, 4], f32, tag=f"a{k}", name=f"a{k}")
            nc.vector.tensor_scalar_mul(out=at[:], in0=ex[:], scalar1=rc[:])
            a_sb.append(at)

        # ---- scaled weights: weff[oi] = a[:,m] * wc rows; wt = weff^T (bf16) ----
        weff = [attn.tile([128, 4 * C], f32, tag=f"weff{oi}", name=f"weff{oi}") for oi in range(2)]
        beff = []
        for oi in range(2):
            for m in range(4):
                nc.vector.tensor_scalar_mul(
                    out=weff[oi][:, m * C : (m + 1) * C],
                    in0=wc_sb[oi][:, m * C : (m + 1) * C],
                    scalar1=a_sb[oi][:, m : m + 1],
                )
            bt = scr.tile([128, 4], f32, tag="btmp", name="btmp")
            be = attn.tile([128, 1], f32, tag=f"beff{oi}", name=f"beff{oi}")
            nc.vector.tensor_mul(bt[:], a_sb[oi][:], bc_sb[oi][:])
            nc.vector.reduce_sum(out=be[:], in_=bt[:], axis=AX)
            beff.append(be)
        wt_sb = [
            attn.tile([128, 4 * C], BF16, tag=f"wt{ci}", name=f"wt{ci}")
            for ci in range(2)
        ]
        for m in range(4):
            for oi in range(2):
                for ci in range(2):
                    psw = psA.tile([128, 128], f32, tag="att", name="att")
                    nc.tensor.transpose(
                        psw[:],
                        weff[oi][:, m * C + ci * 128 : m * C + (ci + 1) * 128],
                        ident[:],
                    )
                    nc.vector.tensor_copy(
                        wt_sb[ci][:, m * C + oi * 128 : m * C + (oi + 1) * 128],
                        psw[:],
                    )

        # ---- pass 2: out[o, s] = sum_{m,c} wt[c, o] * x_m[c, s] (+ beff) ----
        # d-slices in blocks of DG so each stationary weight tile serves
        # 2*DG consecutive matmuls before switching.
        d0 = 0
        while d0 < dhalf:
            dg = min(DG, dhalf - d0)
            xt = {}
            for m in range(4):
                for ci in range(2):
                    for dd_i in range(dg):
                        t = pool2.tile([128, wh], BF16, tag="x2", name="x2")
                        nc.sync.dma_start(
                            out=t[:],
                            in_=xbs[m][
                                ci * 128 : (ci + 1) * 128,
                                (d0 + dd_i) * wh : (d0 + dd_i + 1) * wh,
                            ],
                        )
                        xt[(m, ci, dd_i)] = t
            for oi in range(2):
                pss = {}
                for dd_i in range(dg):
                    for nh in range(n_nh):
                        pss[(dd_i, nh)] = psM.tile(
                            [128, nw], f32, tag="ps", name="ps"
                        )
                for m in range(4):
                    for ci in range(2):
                        wslice = wt_sb[ci][
                            :, m * C + oi * 128 : m * C + (oi + 1) * 128
                        ]
                        for dd_i in range(dg):
                            for nh in range(n_nh):
                                nc.tensor.matmul(
                                    pss[(dd_i, nh)][:],
                                    lhsT=wslice,
                                    rhs=xt[(m, ci, dd_i)][
                                        :, nh * nw : (nh + 1) * nw
                                    ],
                                    start=(m == 0 and ci == 0),
                                    stop=(m == 3 and ci == 1),
                                )
                for dd_i in range(dg):
                    ot = outp.tile([128, wh], BF16, tag="ot", name="ot")
                    for nh in range(n_nh):
                        # drain on the (otherwise idle) Scalar engine:
                        # out = psum + beff, cast to bf16
                        nc.scalar.activation(
                            ot[:, nh * nw : (nh + 1) * nw],
                            pss[(dd_i, nh)][:],
                            AF.Identity,
                            bias=beff[oi][:],
                        )
                    nc.scalar.dma_start(
                        out=out_d[
                            oi * 128 : (oi + 1) * 128,
                            (d0 + dd_i) * wh : (d0 + dd_i + 1) * wh,
                        ],
                        in_=ot[:],
                    )
            d0 += dg
    return nc


_CACHED = {}
LAST_RESULTS = None


def _build(wh=WH, dhalf=DHALF):
    key = (wh, dhalf)
    if key not in _CACHED:
        nc = bacc.Bacc(
            "TRN2",
            target_bir_lowering=False,
            debug=False,
            enable_asserts=False,
            num_devices=NCORES,
        )
        _emit_program(nc, wh=wh, dhalf=dhalf)
        nc.compile()
        _CACHED[key] = nc
    return _CACHED[key]


def _host_prep(Wq, bq, Wk, bk, bc, wh_pool, d):
    """Fold pooling mean + logit scale into augmented [D+1, D] q/k weights."""
    scale_q = 1.0 / (wh_pool * np.sqrt(np.float32(d)))
    wqTaug = np.concatenate(
        [(Wq * scale_q).T, (bq / np.sqrt(np.float32(d)))[None, :]], axis=0
    ).astype(np.float32)
    wkTaug = np.concatenate([(Wk / wh_pool).T, bk[None, :]], axis=0).astype(np.float32)
    bcT = np.ascontiguousarray(bc.T).astype(np.float32)
    ident = np.eye(128, dtype=np.float32)
    return wqTaug, wkTaug, bcT, ident


def kernel(m1, m2, m3, m4, Wq, bq, Wk, bk, Wc, bc, **run_kwargs):
    ms = [np.asarray(x, dtype=np.float32) for x in (m1, m2, m3, m4)]
    Wq, bq, Wk, bk, Wc, bc = (
        np.asarray(x, dtype=np.float32) for x in (Wq, bq, Wk, bk, Wc, bc)
    )
    nc = _build()
    wqTaug, wkTaug, bcT, ident = _host_prep(Wq, bq, Wk, bk, bc, WH, D)
    in_maps = []
    for p in range(NCORES):
        b, h = divmod(p, 2)
        im = {}
        for m in range(4):
            shard = np.ascontiguousarray(
                ms[m][b, :, h * DHALF : (h + 1) * DHALF]
            ).reshape(C, S)
            im[f"x8_{m}"] = shard.astype(NP_FP8)
            im[f"xb_{m}"] = shard.astype(NP_BF16)
        im.update(wqTaug=wqTaug, wkTaug=wkTaug, wc=Wc, bcT=bcT, ident=ident)
        in_maps.append(im)
    global LAST_RESULTS
    res = run_bass_kernel_spmd(
        nc, in_maps, core_ids=list(range(NCORES)), **run_kwargs
    )
    LAST_RESULTS = res
    out = np.empty((B, C, D, W, H), np.float32)
    for p in range(NCORES):
        b, h = divmod(p, 2)
        out[b, :, h * DHALF : (h + 1) * DHALF] = (
            res.results[p]["out"].astype(np.float32).reshape(C, DHALF, W, H)
        )
    return out
